# revision 1
# baseline (speedup 1.0000x reference)
"""Trainium2 Bass kernel for nn_DGN4 (gnn_message_passing).

Reference semantics (B=4, T=2048, D=256, K_SIM=8, K_CON=4):
  xn    = x / max(||x||, 1e-12)                       (row L2-normalize)
  sim   = xn @ xn^T, causally masked (strictly past), masked = -1e9
  A_sim = top-8 per row (one-hot), zeroed outside past
  A_con = "bottom-4" of sim excluding A_sim -- but because masked/future
          entries score +1e9 in the negated space, the reference's con
          picks land on future columns (then zeroed by the causal mask)
          for every row with T - t >= 4.  Only rows T-3..T-1 get
          1, 2, 3 real con-neighbors.  (Verified vs reference.)
  msg_* = degree-normalized mean of selected x rows
  ctx   = alpha*msg_pos + (1-alpha)*msg_neg
  delta = gelu(mix*x + (1-mix)*ctx) * scale   (exact erf gelu; gain/bias per-channel)

Sharding: 8 cores = 4 batches x 2 row-shards.  One *uniform* SPMD program;
per-core differences are data only:
  - program tile k covers row-block OWN[k] with causal width WB[k]*128 cols
    (widths rounded up to 256 so even/odd shards share one program)
  - odd cores get x with adjacent 128-row blocks swapped (an involution), so
    the same static tile offsets address their row blocks; sums over columns
    are permutation invariant
  - the last two 128-col blocks of each tile get a data-driven keep-mask
    (strict-lower-triangular on the diagonal block, ones/zeros elsewhere)

Top-8 per row uses the DVE Max8 instruction (nc.vector.max).  The adjacency
is built as a thresholded mask (w >= tau_8th) scaled by alpha/deg, transposed
128-col-wise on the PE, and aggregated as a PE matmul against x.
"""

import numpy as np

B, T, D = 4, 2048, 256
PB = 128                 # partition block
NBLK = T // PB           # 16 row/col blocks per batch
NTILE = 8                # program tiles per core
# width (in 128-blocks) and own-block index per program tile; widths pair to 18
WB = [2, 16, 4, 14, 6, 12, 8, 10]
OWN = [0, 15, 2, 13, 4, 11, 6, 9]
NEG = -1.0e9
POS = 1.0e9

_PROGRAM = None


def _build_masks():
    """keep-masks for the last two 128-col blocks of each program tile.

    masks[parity][k] is [128, 256] uint8; 1 = keep sim value, 0 = set -1e9.
    Program col-block j holds actual block pi(j) (pi = identity / pair-swap).
    """
    tri = (np.arange(PB)[None, :] < np.arange(PB)[:, None]).astype(np.uint8)
    ones = np.ones((PB, PB), np.uint8)
    zeros = np.zeros((PB, PB), np.uint8)
    out = []
    for parity in (0, 1):
        pi = (lambda j: j) if parity == 0 else (lambda j: j ^ 1)
        m = np.zeros((NTILE, PB, 2 * PB), np.uint8)
        for k in range(NTILE):
            o_act = pi(OWN[k])
            for idx, j in enumerate((WB[k] - 2, WB[k] - 1)):
                a_act = pi(j)
                if a_act < o_act:
                    sub = ones
                elif a_act == o_act:
                    sub = tri
                else:
                    sub = zeros
                m[k][:, idx * PB:(idx + 1) * PB] = sub
        out.append(m)
    return out


def _build_program():
    import concourse.bacc as bacc
    import concourse.tile as tile
    from concourse import mybir

    f32 = mybir.dt.float32
    Alu = mybir.AluOpType
    Act = mybir.ActivationFunctionType

    nc = bacc.Bacc(None)
    x_ext = nc.declare_dram_parameter("x", [T, D], f32, isOutput=False)
    masks_ext = nc.declare_dram_parameter("masks", [NTILE, PB, 2 * PB], mybir.dt.uint8, isOutput=False)
    consts_ext = nc.declare_dram_parameter("consts", [PB, 8], f32, isOutput=False)
    gain_ext = nc.declare_dram_parameter("gain_bc", [PB, D], f32, isOutput=False)
    bias_ext = nc.declare_dram_parameter("bias_bc", [PB, D], f32, isOutput=False)
    eye_ext = nc.declare_dram_parameter("eye", [PB, PB], f32, isOutput=False)
    out_ext = nc.declare_dram_parameter("out", [NTILE * PB, D], f32, isOutput=True)

    with tile.TileContext(nc) as tc:
        with (
            tc.tile_pool(name="singles", bufs=1) as singles,
            tc.tile_pool(name="scr", bufs=3) as scr,
            tc.tile_pool(name="wp", bufs=4) as wp,
            tc.tile_pool(name="ap", bufs=4) as apool,
            tc.tile_pool(name="atp", bufs=4) as atpool,
            tc.tile_pool(name="hip", bufs=1) as hip,
            tc.tile_pool(name="small", bufs=4) as small,
            tc.tile_pool(name="bl", bufs=4) as blp,
            tc.tile_pool(name="ps_sim", bufs=4, space="PSUM") as ps_sim,
            tc.tile_pool(name="ps_t", bufs=2, space="PSUM") as ps_t,
            tc.tile_pool(name="ps_ctx", bufs=2, space="PSUM") as ps_ctx,
        ):
            # ---- load constants / x --------------------------------------
            x_all = singles.tile([PB, NBLK, D], f32)
            x_re = x_ext[:].rearrange("(c p) d -> p c d", p=PB)
            dma_engines = [nc.sync, nc.sync, nc.sync, nc.sync]
            for grp in range(4):
                dma_engines[grp].dma_start(out=x_all[:, grp * 4:(grp + 1) * 4, :],
                                           in_=x_re[:, grp * 4:(grp + 1) * 4, :])
            mask_sb = singles.tile([PB, NTILE, 2 * PB], mybir.dt.uint8)
            nc.sync.dma_start(out=mask_sb, in_=masks_ext[:].rearrange("k p m -> p k m"))
            consts_sb = singles.tile([PB, 8], f32)
            nc.sync.dma_start(out=consts_sb, in_=consts_ext[:])
            gain_sb = singles.tile([PB, D], f32)
            nc.sync.dma_start(out=gain_sb, in_=gain_ext[:])
            bias_sb = singles.tile([PB, D], f32)
            nc.sync.dma_start(out=bias_sb, in_=bias_ext[:])
            eye_sb = singles.tile([PB, PB], f32)
            nc.sync.dma_start(out=eye_sb, in_=eye_ext[:])

            # first-touch copies: the TensorScalar ISA struct encodes only one
            # sync wait, so no TS instruction may be the first on its engine
            # to observe two DMA queues.  Touch every DMA'd tensor on DVE up
            # front (TensorCopy tolerates multiple waits).
            touch_f = singles.tile([PB, 4], f32)
            touch_u = singles.tile([PB, 1], mybir.dt.uint8)
            nc.vector.tensor_copy(touch_f[:, 0:1], x_all[:, 0, 0:1])
            nc.vector.tensor_copy(touch_f[:, 1:2], consts_sb[:, 0:1])
            nc.vector.tensor_copy(touch_f[:, 2:3], gain_sb[:, 0:1])
            nc.vector.tensor_copy(touch_f[:, 3:4], bias_sb[:, 0:1])
            nc.vector.tensor_copy(touch_u, mask_sb[:, 0, 0:1])

            mix_ap = consts_sb[:, 0:1]
            onemmix_ap = consts_sb[:, 1:2]
            alpha_ap = consts_sb[:, 2:3]
            onemalpha_ap = consts_sb[:, 3:4]
            scale_ap = consts_sb[:, 4:5]

            # ---- grouped prologue: norms + normalize + fp32r + transpose --
            # 4 groups of 4 row-blocks, pipelined so the PE's first sim
            # matmuls are unblocked as early as possible.
            f32r = mybir.dt.float32r
            eye_r = singles.tile([PB, PB], f32r)
            nc.vector.tensor_copy(eye_r, eye_sb)
            nrm2 = singles.tile([PB, NBLK], f32)
            nrm = singles.tile([PB, NBLK], f32)
            rinv = singles.tile([PB, NBLK], f32)
            xn_all = singles.tile([PB, NBLK, D], f32)
            x_r = singles.tile([PB, NBLK, D], f32r)
            xnT0 = singles.tile([PB, T], f32)
            xnT1 = singles.tile([PB, T], f32)
            xnT = [xnT0, xnT1]
            # PE first-touch of eye (DMA queue) so real transposes stay
            # within the fused-matmul wait budget
            eye_touch = ps_t.tile([PB, PB], f32, tag="pst")
            nc.tensor.transpose(eye_touch, eye_sb, eye_sb)
            for grp in range(4):
                cs = range(grp * 4, (grp + 1) * 4)
                for c in cs:
                    nc.vector.tensor_copy(x_r[:, c, :], x_all[:, c, :])
                    sq = scr.tile([PB, D], f32, tag="sq")
                    nc.scalar.activation(sq, x_all[:, c, :], Act.Square,
                                         accum_out=nrm2[:, c:c + 1])
                g4 = slice(grp * 4, grp * 4 + 4)
                nc.scalar.activation(nrm[:, g4], nrm2[:, g4], Act.Sqrt)
                nc.vector.tensor_scalar_max(nrm[:, g4], nrm[:, g4], 1e-12)
                nc.vector.reciprocal(rinv[:, g4], nrm[:, g4])
                for c in cs:
                    nc.vector.tensor_scalar_mul(xn_all[:, c, :], x_all[:, c, :],
                                                rinv[:, c:c + 1])
                for g in (grp * 2, grp * 2 + 1):   # two col-blocks per PSUM batch
                    psT = ps_t.tile([PB, 512], f32, tag="pst")
                    for u, (c, hf) in enumerate(
                            [(2 * g, 0), (2 * g + 1, 0), (2 * g, 1), (2 * g + 1, 1)]):
                        nc.tensor.transpose(psT[:, u * PB:(u + 1) * PB],
                                            xn_all[:, c, hf * PB:(hf + 1) * PB], eye_sb)
                    nc.scalar.copy(xnT[0][:, 2 * g * PB:(2 * g + 2) * PB], psT[:, 0:256])
                    nc.vector.tensor_copy(xnT[1][:, 2 * g * PB:(2 * g + 2) * PB], psT[:, 256:512])

            # ---- per row-tile pipeline -----------------------------------
            # emission order: big tiles mid-stream, smallest last so the
            # final serial chain (sim->w->max8->A->AT->agg->blend) is short
            for k in [0, 1, 3, 5, 7, 6, 4, 2]:
                nb = WB[k]
                W = nb * PB
                own = OWN[k]
                w_t = wp.tile([PB, W], f32, tag="w")

                # sim = xn_tile @ xnT  (fp32, K=256 via two 128-chunks)
                n512 = (W + 511) // 512
                for j in range(n512):
                    lo = j * 512
                    n = min(512, W - lo)
                    ps = ps_sim.tile([PB, n], f32, tag="ps_sim")
                    for hf in (0, 1):
                        nc.tensor.matmul(
                            ps,
                            xnT[hf][:, own * PB:(own + 1) * PB],
                            xnT[hf][:, lo:lo + n],
                            start=(hf == 0), stop=(hf == 1))
                    # copy to w, masking the last 256 cols via keep-mask
                    mstart = W - 256
                    if lo + n <= mstart:
                        nc.scalar.copy(w_t[:, lo:lo + n], ps)
                    else:
                        if lo < mstart:
                            nc.scalar.copy(w_t[:, lo:mstart], ps[:, 0:mstart - lo])
                        off = max(0, mstart - lo)
                        nc.vector.memset(w_t[:, mstart:W], NEG)
                        nc.vector.copy_predicated(w_t[:, mstart:W],
                                                  mask_sb[:, k, :], ps[:, off:off + 256])

                # top-8 values per row (HW Max8), threshold + degree
                v8 = small.tile([PB, 8], f32, tag="v8")
                nc.vector.max(out=v8, in_=w_t)
                tau = small.tile([PB, 1], f32, tag="tau")
                nc.vector.tensor_scalar_max(tau, v8[:, 7:8], -1e8)
                cnt8 = small.tile([PB, 8], f32, tag="cnt8")
                deg = small.tile([PB, 1], f32, tag="deg")
                nc.vector.tensor_scalar(cnt8, v8, -1e8, None, op0=Alu.is_gt,
                                        op1=Alu.add, accum_out=deg)
                nc.vector.tensor_scalar_max(deg, deg, 1.0)
                coef = small.tile([PB, 1], f32, tag="coef")
                nc.vector.reciprocal(coef, deg)
                nc.vector.tensor_scalar(coef, coef, alpha_ap, None, op0=Alu.mult)

                # A_scaled = (w >= tau) * (alpha/deg)
                A_t = apool.tile([PB, W], f32, tag="A")
                # first-touch so the TS write below inherits no cross-engine
                # slot-reuse wait (TS encodes a single wait)
                nc.vector.memset(A_t[:, 0:1], 0.0)
                nc.vector.scalar_tensor_tensor(
                    A_t, w_t, tau, coef.to_broadcast([PB, W]),
                    op0=Alu.is_ge, op1=Alu.mult)

                # contrarian branch: only the full-width tile can have any
                # (rows T-3..T-1); exact zeros elsewhere by construction.
                Ac_t = None
                if k == 1:
                    w2 = wp.tile([PB, W], f32, tag="w")
                    nc.vector.match_replace(out=w2, in_to_replace=v8,
                                            in_values=w_t, imm_value=POS)
                    nc.vector.tensor_scalar_mul(w2, w2, -1.0)   # v2 = -w2
                    vc8 = small.tile([PB, 8], f32, tag="vc8")
                    nc.vector.max(out=vc8, in_=w2)
                    hi_t = hip.tile([PB, W], f32, tag="hi")
                    nc.gpsimd.tensor_scalar(hi_t, w2, 1e8, None, op0=Alu.is_lt)
                    cnt4 = small.tile([PB, 4], f32, tag="cnt4")
                    degc = small.tile([PB, 1], f32, tag="degc")
                    # cols 0..3 always hold futures(+1e9) or reals (never the
                    # -1e9 replaced-A_sim sentinels: >=4 entries above -1e9
                    # always exist), so "real" == (v < 1e8).
                    nc.vector.tensor_scalar(cnt4, vc8[:, 0:4], 1e8, None,
                                            op0=Alu.is_lt, op1=Alu.add,
                                            accum_out=degc)
                    nc.vector.tensor_scalar_max(degc, degc, 1.0)
                    coefc = small.tile([PB, 1], f32, tag="coefc")
                    nc.vector.reciprocal(coefc, degc)
                    nc.vector.tensor_scalar(coefc, coefc, onemalpha_ap, None,
                                            op0=Alu.mult)
                    Ac_t = apool.tile([PB, W], f32, tag="A")
                    nc.vector.memset(Ac_t[:, 0:1], 0.0)
                    nc.vector.scalar_tensor_tensor(
                        Ac_t, w2, vc8[:, 3:4], hi_t,
                        op0=Alu.is_ge, op1=Alu.mult)
                    nc.vector.tensor_scalar(Ac_t, Ac_t, coefc, None, op0=Alu.mult)

                # transpose A (and Ac) 128-col-wise on the PE
                def transpose_to(src, dst, nblocks):
                    for gi, g in enumerate(range(0, nblocks, 4)):
                        cnt = min(4, nblocks - g)
                        psT = ps_t.tile([PB, 512], f32, tag="pst")
                        for u in range(cnt):
                            nc.tensor.transpose(
                                psT[:, u * PB:(u + 1) * PB],
                                src[:, (g + u) * PB:(g + u + 1) * PB], eye_sb)
                        nc.scalar.copy(dst[:, g * PB:(g + cnt) * PB],
                                       psT[:, 0:cnt * PB])

                AT_t = atpool.tile([PB, W], f32r, tag="AT")
                transpose_to(A_t, AT_t, nb)
                if Ac_t is not None:
                    ATc_t = atpool.tile([PB, W], f32r, tag="AT")
                    transpose_to(Ac_t, ATc_t, nb)

                # ctx = sum_s A_scaled[t,s] * x[s,:]  (PE, accumulate in PSUM)
                ctx_ps = ps_ctx.tile([PB, D], f32, tag="ctx")
                last = nb - 1 if Ac_t is None else None
                for c in range(nb):
                    nc.tensor.matmul(ctx_ps, AT_t[:, c * PB:(c + 1) * PB],
                                     x_r[:, c, :],
                                     start=(c == 0), stop=(c == last))
                if Ac_t is not None:
                    for c in range(nb):
                        nc.tensor.matmul(ctx_ps, ATc_t[:, c * PB:(c + 1) * PB],
                                         x_r[:, c, :],
                                         start=False, stop=(c == nb - 1))

                # blend + gelu + scale
                xp = blp.tile([PB, D], f32, tag="xp")
                nc.vector.tensor_scalar(xp, x_all[:, own, :], mix_ap, None,
                                        op0=Alu.mult)
                bl = blp.tile([PB, D], f32, tag="blv")
                nc.vector.scalar_tensor_tensor(bl, ctx_ps, onemmix_ap, xp,
                                               op0=Alu.mult, op1=Alu.add)
                z_t = blp.tile([PB, D], f32, tag="z")
                nc.vector.tensor_mul(z_t, bl, gain_sb)
                nc.vector.tensor_add(z_t, z_t, bias_sb)
                g_t = blp.tile([PB, D], f32, tag="g")
                nc.scalar.activation(g_t, z_t, Act.Gelu)
                d_t = blp.tile([PB, D], f32, tag="d")
                nc.vector.memset(d_t[:, 0:1], 0.0)
                nc.vector.tensor_scalar(d_t, g_t, scale_ap, None, op0=Alu.mult)
                out_eng = nc.sync
                out_eng.dma_start(out=out_ext[k * PB:(k + 1) * PB, :], in_=d_t)

    # Bacc lowering: splits multi-waits into event semaphores / nops so every
    # HW instruction respects its 1-wait encoding limit.
    nc.compile()
    return nc


def _get_program():
    global _PROGRAM
    if _PROGRAM is None:
        _PROGRAM = _build_program()
    return _PROGRAM


def kernel(**inputs):
    x = np.ascontiguousarray(np.asarray(inputs["x"], dtype=np.float32))
    gain = np.asarray(inputs["gain"], dtype=np.float32).reshape(D)
    bias = np.asarray(inputs["bias"], dtype=np.float32).reshape(D)
    log_mix = float(np.asarray(inputs["log_mix"]))
    log_alpha = float(np.asarray(inputs["log_alpha"]))
    log_scale = float(np.asarray(inputs["log_scale"]))

    mix = np.float32(1.0 / (1.0 + np.exp(-np.float64(log_mix))))
    alpha = np.float32(1.0 / (1.0 + np.exp(-np.float64(log_alpha))))
    scale = np.float32(np.logaddexp(0.0, np.float64(log_scale)) + 0.01)

    consts = np.zeros((PB, 8), np.float32)
    consts[:, 0] = mix
    consts[:, 1] = np.float32(1.0) - mix
    consts[:, 2] = alpha
    consts[:, 3] = np.float32(1.0) - alpha
    consts[:, 4] = scale
    gain_bc = np.ascontiguousarray(np.broadcast_to(gain[None, :], (PB, D)))
    bias_bc = np.ascontiguousarray(np.broadcast_to(bias[None, :], (PB, D)))
    eye = np.eye(PB, dtype=np.float32)
    masks = _build_masks()

    swap_perm = np.arange(NBLK).reshape(-1, 2)[:, ::-1].reshape(-1)

    in_maps = []
    for c in range(8):
        b, p = c // 2, c % 2
        xb = x[b]
        if p:
            xb = np.ascontiguousarray(
                xb.reshape(NBLK, PB, D)[swap_perm].reshape(T, D))
        in_maps.append({
            "x": xb,
            "masks": masks[p],
            "consts": consts,
            "gain_bc": gain_bc,
            "bias_bc": bias_bc,
            "eye": eye,
        })

    from concourse.bass_utils import run_bass_kernel_spmd
    nc = _get_program()
    res = run_bass_kernel_spmd(nc, in_maps, list(range(8))).results

    out = np.empty((B, T, D), np.float32)
    for c in range(8):
        b, p = c // 2, c % 2
        o = np.asarray(res[c]["out"])
        for k in range(NTILE):
            g_act = OWN[k] ^ p
            out[b, g_act * PB:(g_act + 1) * PB, :] = o[k * PB:(k + 1) * PB, :]
    return out



# revision 12
# speedup vs baseline: 1.4116x; 1.4116x over previous
"""Trainium2 Bass kernel for nn_DGN4 (gnn_message_passing)  -- v2.

Reference semantics (B=4, T=2048, D=256, K_SIM=8, K_CON=4):
  xn    = x / max(||x||, 1e-12)                       (row L2-normalize)
  sim   = xn @ xn^T, causally masked (strictly past), masked = -1e9
  A_sim = top-8 per row (one-hot), zeroed outside past
  A_con = "bottom-4" of sim excluding A_sim -- because masked/future
          entries score +1e9 in the negated space, the reference's con
          picks land on future columns (then zeroed by the causal mask)
          for every row with T - t >= 4.  Only rows T-3..T-1 get
          1..3 real con-neighbors.
  msg_* = degree-normalized mean of selected x rows
  ctx   = alpha*msg_pos + (1-alpha)*msg_neg
  delta = gelu(mix*x + (1-mix)*ctx) * scale   (exact erf gelu; per-channel
          gain/bias fold is skipped when gain==1, bias==0 -- the common case)

Sharding: 8 cores = 4 batches x 2 row-shards, one uniform SPMD program;
per-core differences are data only (odd cores get adjacent 128-row blocks
swapped so the same static tile offsets address their rows).

v2 performance notes (driven by the CoreSim cost model):
  - x is shipped as bf16: halves the serialized input-DMA time and makes
    every PE op (sim matmuls, transposes, aggregation) run at 1 cycle/row
    instead of fp32's 4, and DVE elementwise ops eligible for 2x modes.
  - the causal keep-mask is injected into PSUM by the PE itself:
    matmul(ps_tail, eye_bf, penalty_pattern, start=False) adds the 0/-1e9
    pattern onto the sim accumulation, so the psum->w copies are plain.
  - engine rebalance: psum->w copies on Act, top-8 on DVE, adjacency
    build (threshold stt) on Pool/gpsimd, transposes+aggregation on PE.
  - stage-skewed emission: sim(tile i) | select(tile i-1) | agg(tile i-2)
    so no engine stalls on the select chain of its own tile.
  - selection thresholds stay fp32 (w tile) -- bf16 sim values would tie
    at the 8th-largest and break the degree normalization.
"""

import numpy as np

B, T, D = 4, 2048, 256
PB = 128                 # partition block
NBLK = T // PB           # 16 row/col blocks per batch
NTILE = 8                # program tiles per core
# width (in 128-blocks) and own-block index per program tile; widths pair to 18
WB = [2, 16, 4, 14, 6, 12, 8, 10]
OWN = [0, 15, 2, 13, 4, 11, 6, 9]
NEG = -1.0e9
NEGF = -60000.0
POS = 1.0e9

# tile processing order (pipeline): big tiles early-mid, smallest last
TORD = [0, 1, 3, 5, 7, 6, 4, 2]

_PROGRAMS = {}


def _build_patterns():
    """Penalty patterns (0 = keep, NEG = masked) for the last two 128-col
    blocks of each program tile, as a function of tile parity (k%2) and
    core parity.  patterns[parity] is [2, PB, 2*PB] float32."""
    tri = np.where(np.arange(PB)[None, :] < np.arange(PB)[:, None], 0.0, NEGF)
    keep = np.zeros((PB, PB), np.float32)
    mask = np.full((PB, PB), NEGF, np.float32)
    out = []
    for parity in (0, 1):
        m = np.zeros((2, PB, 2 * PB), np.float32)
        if parity == 0:
            m[0] = np.concatenate([tri, mask], axis=1)   # even k
            m[1] = np.concatenate([keep, tri], axis=1)   # odd k
        else:
            m[0] = np.concatenate([tri, keep], axis=1)
            m[1] = np.concatenate([mask, tri], axis=1)
        out.append(m.astype(np.float32))
    return out


def _build_program(unit_affine=True):
    import concourse.bacc as bacc
    import concourse.tile as tile
    from concourse import mybir

    f32 = mybir.dt.float32
    f16 = mybir.dt.float16
    Alu = mybir.AluOpType
    Act = mybir.ActivationFunctionType

    nc = bacc.Bacc(None)
    x_ext = nc.declare_dram_parameter("x", [T, D], f16, isOutput=False)
    pat_ext = nc.declare_dram_parameter("patterns", [2, PB, 2 * PB], f16, isOutput=False)
    consts_ext = nc.declare_dram_parameter("consts", [PB, 8], f32, isOutput=False)
    eye_ext = nc.declare_dram_parameter("eye_bf", [PB, PB], f16, isOutput=False)
    if not unit_affine:
        gain_ext = nc.declare_dram_parameter("gain_bc", [PB, D], f32, isOutput=False)
        bias_ext = nc.declare_dram_parameter("bias_bc", [PB, D], f32, isOutput=False)
    out_ext = nc.declare_dram_parameter("out", [NTILE * PB, D], f32, isOutput=True)

    with tile.TileContext(nc) as tc:
        with (
            tc.tile_pool(name="singles", bufs=1) as singles,
            tc.tile_pool(name="scr", bufs=4) as scr,
            tc.tile_pool(name="wp", bufs=3) as wp,
            tc.tile_pool(name="t1p", bufs=3) as t1p,
            tc.tile_pool(name="ap", bufs=3) as apool,
            tc.tile_pool(name="atp", bufs=3) as atpool,
            tc.tile_pool(name="small", bufs=6) as small,
            tc.tile_pool(name="bl", bufs=4) as blp,
            tc.tile_pool(name="ps_sim", bufs=4, space="PSUM") as ps_sim,
            tc.tile_pool(name="ps_t", bufs=2, space="PSUM") as ps_t,
            tc.tile_pool(name="ps_ctx", bufs=2, space="PSUM") as ps_ctx,
        ):
            # ---- input DMAs (transfers serialize on the DMA engines; order
            # matters: first blocks + patterns first so compute starts early)
            x_all = singles.tile([PB, NBLK, D], f16)
            x_re = x_ext[:].rearrange("(c p) d -> p c d", p=PB)
            nc.sync.dma_start(out=x_all[:, 0:2, :], in_=x_re[:, 0:2, :])
            pat_sb = singles.tile([PB, 2, 2 * PB], f16)
            nc.sync.dma_start(out=pat_sb, in_=pat_ext[:].rearrange("q p m -> p q m"))
            eye_sb = singles.tile([PB, PB], f16)
            nc.sync.dma_start(out=eye_sb, in_=eye_ext[:])
            consts_sb = singles.tile([PB, 8], f32)
            nc.sync.dma_start(out=consts_sb, in_=consts_ext[:])
            for g in range(1, 8):
                nc.sync.dma_start(out=x_all[:, 2 * g:2 * g + 2, :],
                                  in_=x_re[:, 2 * g:2 * g + 2, :])
            if not unit_affine:
                gain_sb = singles.tile([PB, D], f32)
                nc.sync.dma_start(out=gain_sb, in_=gain_ext[:])
                bias_sb = singles.tile([PB, D], f32)
                nc.sync.dma_start(out=bias_sb, in_=bias_ext[:])

            # first-touch copies: TensorScalar-family instructions encode only
            # one sync wait, so no TS op may be the first on its engine to
            # observe two DMA queues.  TensorCopy tolerates multiple waits.
            touch_b = singles.tile([PB, 4], f16)
            touch_f = singles.tile([PB, 2], f32)
            nc.vector.tensor_copy(touch_b[:, 0:1], x_all[:, 0, 0:1])
            nc.vector.tensor_copy(touch_b[:, 1:2], pat_sb[:, 0, 0:1])
            nc.vector.tensor_copy(touch_b[:, 2:3], eye_sb[:, 0:1])
            nc.vector.tensor_copy(touch_f[:, 0:1], consts_sb[:, 0:1])
            touch_p = singles.tile([PB, 2], f16)
            nc.gpsimd.tensor_copy(touch_p[:, 0:1], x_all[:, 0, 0:1])
            nc.gpsimd.tensor_copy(touch_p[:, 1:2], pat_sb[:, 0, 0:1])

            mix_ap = consts_sb[:, 0:1]
            alpha1m_ap = consts_sb[:, 2:3]      # alpha*(1-mix)
            onemalpha1m_ap = consts_sb[:, 3:4]  # (1-alpha)*(1-mix)
            scale_ap = consts_sb[:, 4:5]

            # PE first-touch of eye (DMA queue) so real transposes stay
            # within the fused-matmul wait budget
            eye_touch = ps_t.tile([PB, PB], f16, tag="pst")
            nc.tensor.transpose(eye_touch, eye_sb, eye_sb)

            # ---- grouped prologue: norms + normalize + transpose ---------
            nrm2 = singles.tile([PB, NBLK], f32)
            nrm = singles.tile([PB, NBLK], f32)
            rinv = singles.tile([PB, NBLK], f32)
            xn_all = singles.tile([PB, NBLK, D], f16)
            xnT0 = singles.tile([PB, T], f16)
            xnT1 = singles.tile([PB, T], f16)
            xnT = [xnT0, xnT1]
            for grp in range(4):
                cs = range(grp * 4, (grp + 1) * 4)
                for c in cs:
                    # norm^2: split across engines (DVE stt / Act Square)
                    if c % 2 == 0:
                        sq = scr.tile([PB, D], f16, tag="sq")
                        nc.vector.scalar_tensor_tensor(
                            sq, x_all[:, c, :], 1.0, x_all[:, c, :],
                            op0=Alu.mult, op1=Alu.mult,
                            accum_out=nrm2[:, c:c + 1])
                    else:
                        sqf = scr.tile([PB, D], f32, tag="sqf")
                        nc.scalar.activation(sqf, x_all[:, c, :], Act.Square,
                                             accum_out=nrm2[:, c:c + 1])
                g4 = slice(grp * 4, grp * 4 + 4)
                nc.scalar.activation(nrm[:, g4], nrm2[:, g4], Act.Sqrt)
                nc.vector.tensor_scalar_max(nrm[:, g4], nrm[:, g4], 1e-12)
                nc.vector.reciprocal(rinv[:, g4], nrm[:, g4])
                for c in cs:
                    nc.vector.tensor_scalar_mul(xn_all[:, c, :], x_all[:, c, :],
                                                rinv[:, c:c + 1])
                # two psT batches per group: one per feature half
                for hf in (0, 1):
                    psT = ps_t.tile([PB, 512], f16, tag="pst")
                    for u, c in enumerate(cs):
                        nc.tensor.transpose(
                            psT[:, u * PB:(u + 1) * PB],
                            xn_all[:, c, hf * PB:(hf + 1) * PB], eye_sb)
                    dst = xnT[hf][:, grp * 4 * PB:(grp * 4 + 4) * PB]
                    if hf == 0:
                        nc.vector.tensor_copy(dst, psT)
                    else:
                        nc.scalar.copy(dst, psT)

            # ---- per-tile pipeline stages --------------------------------
            state = {}

            def stage_sim(k):
                nb = WB[k]
                W = nb * PB
                own = OWN[k]
                w_t = wp.tile([PB, W], f32, tag="w")
                n512 = (W + 511) // 512
                for j in range(n512):
                    lo = j * 512
                    n = min(512, W - lo)
                    ps = ps_sim.tile([PB, n], f32, tag="ps_sim")
                    last_chunk = (j == n512 - 1)
                    nc.tensor.matmul(
                        ps, xnT[0][:, own * PB:(own + 1) * PB],
                        xnT[0][:, lo:lo + n], start=True, stop=False)
                    if last_chunk:
                        # PE adds the causal penalty pattern onto the tail:
                        # eye^T @ pattern == pattern
                        nc.tensor.matmul(
                            ps[:, n - 256:n], eye_sb, pat_sb[:, k % 2, :],
                            start=False, stop=False, skip_group_check=True)
                    nc.tensor.matmul(
                        ps, xnT[1][:, own * PB:(own + 1) * PB],
                        xnT[1][:, lo:lo + n], start=False, stop=True)
                    nc.scalar.copy(w_t[:, lo:lo + n], ps)
                state[k] = {"w": w_t}

            def stage_select(k):
                nb = WB[k]
                W = nb * PB
                st = state[k]
                w_t = st["w"]
                v8 = small.tile([PB, 8], f32, tag="v8")
                nc.vector.max(out=v8, in_=w_t)
                tau = small.tile([PB, 1], f32, tag="tau")
                nc.vector.tensor_scalar_max(tau, v8[:, 7:8], -1e4)
                cnt8 = small.tile([PB, 8], f32, tag="cnt8")
                deg = small.tile([PB, 1], f32, tag="deg")
                nc.vector.tensor_scalar(cnt8, v8, -1e4, None, op0=Alu.is_gt,
                                        op1=Alu.add, accum_out=deg)
                nc.vector.tensor_scalar_max(deg, deg, 1.0)
                coef = small.tile([PB, 1], f32, tag="coef")
                nc.vector.reciprocal(coef, deg)
                nc.vector.tensor_scalar(coef, coef, alpha1m_ap, None, op0=Alu.mult)

                # A_scaled = (w >= tau) * (alpha*(1-mix)/deg), f16
                A_t = apool.tile([PB, W], f16, tag="A")
                nc.gpsimd.memzero(A_t[:, 0:2])
                nc.gpsimd.tensor_scalar(A_t, w_t, tau, coef,
                                        op0=Alu.is_ge, op1=Alu.mult)
                st["A"] = A_t

                # contrarian branch: only the full-width tile can have any
                # (rows T-3..T-1); exact zeros elsewhere by construction.
                if k == 1:
                    wneg = t1p.tile([PB, W], f32, tag="t1")
                    nc.gpsimd.tensor_scalar_mul(wneg, w_t, -1.0)
                    negv8 = small.tile([PB, 8], f32, tag="negv8")
                    nc.vector.tensor_scalar_mul(negv8, v8, -1.0)
                    w2 = t1p.tile([PB, W], f32, tag="t1")
                    nc.vector.match_replace(out=w2, in_to_replace=negv8,
                                            in_values=wneg, imm_value=NEG)
                    vc8 = small.tile([PB, 8], f32, tag="vc8")
                    nc.vector.max(out=vc8, in_=w2)
                    cnt4 = small.tile([PB, 4], f32, tag="cnt4")
                    degc = small.tile([PB, 1], f32, tag="degc")
                    nc.vector.tensor_scalar(cnt4, vc8[:, 0:4], 1e4, None,
                                            op0=Alu.is_lt, op1=Alu.add,
                                            accum_out=degc)
                    nc.vector.tensor_scalar_max(degc, degc, 1.0)
                    coefc = small.tile([PB, 1], f32, tag="coefc")
                    nc.vector.reciprocal(coefc, degc)
                    nc.vector.tensor_scalar(coefc, coefc, onemalpha1m_ap, None,
                                            op0=Alu.mult)
                    # hi = (w2 < 1e8) * coefc   (zero for future sentinels)
                    hi_t = t1p.tile([PB, W], f32, tag="t1")
                    nc.gpsimd.tensor_scalar(hi_t, w2, 1e4, coefc,
                                            op0=Alu.is_lt, op1=Alu.mult)
                    Ac_t = apool.tile([PB, W], f16, tag="A")
                    nc.vector.scalar_tensor_tensor(
                        Ac_t, w2, vc8[:, 3:4], hi_t,
                        op0=Alu.is_ge, op1=Alu.mult)
                    st["Ac"] = Ac_t

            def transpose_to(src, dst, nblocks, copy_eng_iter):
                for g in range(0, nblocks, 4):
                    cnt = min(4, nblocks - g)
                    psT = ps_t.tile([PB, 512], f16, tag="pst")
                    for u in range(cnt):
                        nc.tensor.transpose(
                            psT[:, u * PB:(u + 1) * PB],
                            src[:, (g + u) * PB:(g + u + 1) * PB], eye_sb)
                    eng = next(copy_eng_iter)
                    if eng == "v":
                        nc.vector.tensor_copy(dst[:, g * PB:(g + cnt) * PB],
                                              psT[:, 0:cnt * PB])
                    else:
                        nc.scalar.copy(dst[:, g * PB:(g + cnt) * PB],
                                       psT[:, 0:cnt * PB])

            def _alternator():
                while True:
                    yield "v"
                    yield "s"

            copy_eng = _alternator()

            def stage_agg(k):
                nb = WB[k]
                W = nb * PB
                own = OWN[k]
                st = state[k]
                A_t = st["A"]
                Ac_t = st.get("Ac")

                AT_t = atpool.tile([PB, W], f16, tag="AT")
                transpose_to(A_t, AT_t, nb, copy_eng)
                if Ac_t is not None:
                    ATc_t = atpool.tile([PB, W], f16, tag="AT")
                    transpose_to(Ac_t, ATc_t, nb, copy_eng)

                ctx_ps = ps_ctx.tile([PB, D], f32, tag="ctx")
                last = nb - 1 if Ac_t is None else None
                for c in range(nb):
                    nc.tensor.matmul(ctx_ps, AT_t[:, c * PB:(c + 1) * PB],
                                     x_all[:, c, :],
                                     start=(c == 0), stop=(c == last))
                if Ac_t is not None:
                    for c in range(nb):
                        nc.tensor.matmul(ctx_ps, ATc_t[:, c * PB:(c + 1) * PB],
                                         x_all[:, c, :],
                                         start=False, stop=(c == nb - 1))

                # blend + gelu + scale  (ctx_ps is already (1-mix)-scaled via
                # the adjacency coefficients)
                bl = blp.tile([PB, D], f32, tag="blv")
                nc.vector.scalar_tensor_tensor(bl, x_all[:, own, :], mix_ap,
                                               ctx_ps, op0=Alu.mult, op1=Alu.add)
                z_t = bl
                if not unit_affine:
                    z_t = blp.tile([PB, D], f32, tag="z")
                    nc.vector.tensor_mul(z_t, bl, gain_sb)
                    nc.vector.tensor_add(z_t, z_t, bias_sb)
                g_t = blp.tile([PB, D], f32, tag="g")
                nc.scalar.activation(g_t, z_t, Act.Gelu)
                d_t = blp.tile([PB, D], f32, tag="d")
                nc.vector.memset(d_t[:, 0:1], 0.0)
                nc.vector.tensor_scalar(d_t, g_t, scale_ap, None, op0=Alu.mult)
                nc.sync.dma_start(out=out_ext[k * PB:(k + 1) * PB, :], in_=d_t)
                del state[k]

            # stage-skewed emission: sim(i) | select(i-1) | agg(i-2)
            for i in range(len(TORD) + 2):
                if i < len(TORD):
                    stage_sim(TORD[i])
                if 1 <= i <= len(TORD):
                    stage_select(TORD[i - 1])
                if i >= 2:
                    stage_agg(TORD[i - 2])

    nc.compile()
    return nc


def _get_program(unit_affine=True):
    key = bool(unit_affine)
    if key not in _PROGRAMS:
        _PROGRAMS[key] = _build_program(unit_affine=key)
    return _PROGRAMS[key]


def _make_in_maps(inputs):
    """Host-side prep: returns (in_maps for cores 0-7, unit_affine flag)."""
    x = np.asarray(inputs["x"], dtype=np.float32)
    gain = np.asarray(inputs["gain"], dtype=np.float32).reshape(D)
    bias = np.asarray(inputs["bias"], dtype=np.float32).reshape(D)
    log_mix = float(np.asarray(inputs["log_mix"]))
    log_alpha = float(np.asarray(inputs["log_alpha"]))
    log_scale = float(np.asarray(inputs["log_scale"]))

    mix = np.float32(1.0 / (1.0 + np.exp(-np.float64(log_mix))))
    alpha = np.float32(1.0 / (1.0 + np.exp(-np.float64(log_alpha))))
    scale = np.float32(np.logaddexp(0.0, np.float64(log_scale)) + 0.01)
    unit_affine = bool(np.all(gain == 1.0) and np.all(bias == 0.0))

    consts = np.zeros((PB, 8), np.float32)
    consts[:, 0] = mix
    consts[:, 1] = np.float32(1.0) - mix
    consts[:, 2] = alpha * (np.float32(1.0) - mix)
    consts[:, 3] = (np.float32(1.0) - alpha) * (np.float32(1.0) - mix)
    consts[:, 4] = scale
    eye_bf = np.eye(PB, dtype=np.float32).astype(np.float16)
    patterns = _build_patterns()

    swap_perm = np.arange(NBLK).reshape(-1, 2)[:, ::-1].reshape(-1)

    in_maps = []
    for c in range(8):
        b, p = c // 2, c % 2
        xb = x[b]
        if p:
            xb = xb.reshape(NBLK, PB, D)[swap_perm].reshape(T, D)
        im = {
            "x": np.ascontiguousarray(xb.astype(np.float16)),
            "patterns": np.ascontiguousarray(
                patterns[p].astype(np.float16)),
            "consts": consts,
            "eye_bf": eye_bf,
        }
        if not unit_affine:
            im["gain_bc"] = np.ascontiguousarray(
                np.broadcast_to(gain[None, :], (PB, D)).astype(np.float32))
            im["bias_bc"] = np.ascontiguousarray(
                np.broadcast_to(bias[None, :], (PB, D)).astype(np.float32))
        in_maps.append(im)
    return in_maps, unit_affine


def kernel(**inputs):
    in_maps, unit_affine = _make_in_maps(inputs)
    from concourse.bass_utils import run_bass_kernel_spmd
    nc = _get_program(unit_affine)
    res = run_bass_kernel_spmd(nc, in_maps, list(range(8))).results

    out = np.empty((B, T, D), np.float32)
    for c in range(8):
        b, p = c // 2, c % 2
        o = np.asarray(res[c]["out"])
        for k in range(NTILE):
            g_act = OWN[k] ^ p
            out[b, g_act * PB:(g_act + 1) * PB, :] = o[k * PB:(k + 1) * PB, :]
    return out
